# revision 1
# baseline (speedup 1.0000x reference)
"""Trainium2 Bass kernel for BiLinearSigmoidAttention.

Reference math (per batch b, with L = length[b]):
    qn = l2norm(query), cn = l2norm(context)
    raw[q,k] = qn[q] . cn[k]            (masked: k >= L -> -1e30)
    sig = sigmoid(raw)
    den[q] = max(sum_k sig[q,k], 1)
    scores[q,k] = sig[q,k] / den[q]     (rows q >= L zeroed)
    att[q,:] = sum_k scores[q,k] * context[k,:]
    out = concat([qn, att], -1)
returns (out [B,S,2D], scores [B,S,S])

Device mapping (8 NeuronCores, pure data parallel over B=32 -> 4 per core):
  - mm1 computes scoresT [k_part, q_free] so the length mask is a
    per-partition bias fused into the ACT sigmoid.
  - denominator = ones-column matmuls sharing mm2's loaded weights,
    accumulated per q-block into tiny PSUM tiles (partition-major).
  - scores output produced by PE transposes of sigT, scaled by
    w = qmask/den during PSUM->SBUF eviction.
  - matmuls run as float32r (full-rate fp32); transposes as fp32.
"""

import numpy as np

import concourse.bacc as bacc
import concourse.mybir as mybir
import concourse.tile as tile
from concourse.bass_utils import run_bass_kernel_spmd

B, S, D = 32, 1024, 512
NCORES = 8
BPC = B // NCORES          # batches per core
P = 128                    # partitions
NT = S // P                # 8 s-tiles
ND = D // P                # 4 d-chunks
NEG = np.float32(-1e30)

F32 = mybir.dt.float32
F32R = mybir.dt.float32r
AF = mybir.ActivationFunctionType
ALU = mybir.AluOpType
AX = mybir.AxisListType


def _r(ap):
    """View an fp32 AP as float32r for full-rate PE matmuls."""
    return ap.bitcast(F32R)


def build_kernel():
    nc = bacc.Bacc("TRN2", target_bir_lowering=False, debug=False)

    q_d = nc.dram_tensor("query", [BPC, S, D], F32, kind="ExternalInput")
    c_d = nc.dram_tensor("context", [BPC, S, D], F32R, kind="ExternalInput")
    # keybias[b, p, kt] = 0 if kt*P+p < L else -1e30
    kb_d = nc.dram_tensor("keybias", [BPC, P, NT], F32, kind="ExternalInput")
    # qmask[b, p, qb] = 1 if qb*P+p < L else 0
    qm_d = nc.dram_tensor("qmask", [BPC, P, NT], F32, kind="ExternalInput")
    id_d = nc.dram_tensor("identity", [P, P], F32, kind="ExternalInput")
    idr_d = nc.dram_tensor("identity_r", [P, P], F32R, kind="ExternalInput")
    on_d = nc.dram_tensor("ones", [P, 2], F32R, kind="ExternalInput")
    out_d = nc.dram_tensor("out", [BPC, S, 2 * D], F32, kind="ExternalOutput")
    sc_d = nc.dram_tensor("scores", [BPC, S, S], F32, kind="ExternalOutput")

    with tile.TileContext(nc) as tc:
        _body(tc, q_d, c_d, kb_d, qm_d, id_d, idr_d, on_d, out_d, sc_d)
    nc.compile()
    return nc


def _body(tc, q_d, c_d, kb_d, qm_d, id_d, idr_d, on_d, out_d, sc_d):
    import os

    PHASE = int(os.environ.get("KERNEL_PHASE", "4"))
    nc = tc.nc
    from contextlib import ExitStack

    ctx = ExitStack()
    with ctx:
        const = ctx.enter_context(tc.tile_pool(name="const", bufs=1))
        qpool = ctx.enter_context(tc.tile_pool(name="q", bufs=2))
        cpool = ctx.enter_context(tc.tile_pool(name="c", bufs=2))
        tpool = ctx.enter_context(tc.tile_pool(name="t", bufs=1))
        sgpool = ctx.enter_context(tc.tile_pool(name="sg", bufs=1))
        mpool = ctx.enter_context(tc.tile_pool(name="m", bufs=2))
        spool = ctx.enter_context(tc.tile_pool(name="s", bufs=3))
        opool = ctx.enter_context(tc.tile_pool(name="o", bufs=3))
        ps1 = ctx.enter_context(tc.tile_pool(name="ps1", bufs=2, space="PSUM"))
        pst = ctx.enter_context(tc.tile_pool(name="pst", bufs=2, space="PSUM"))
        ps2 = ctx.enter_context(tc.tile_pool(name="ps2", bufs=2, space="PSUM"))
        psd = ctx.enter_context(tc.tile_pool(name="psd", bufs=2, space="PSUM"))

        ident = const.tile([P, P], F32, tag="ident")
        identr = const.tile([P, P], F32R, tag="identr")
        ones = const.tile([P, 2], F32R, tag="ones")
        nc.sync.dma_start(ident[:], id_d[:])
        nc.sync.dma_start(identr[:], idr_d[:])
        nc.sync.dma_start(ones[:], on_d[:])

        for b in range(BPC):
            # ---- load ----
            qt = qpool.tile([P, NT, D], F32, tag="qt")       # qn (in-place)
            ct = cpool.tile([P, NT, D], F32R, tag="ct")       # raw context
            kb = mpool.tile([P, NT], F32, tag="kb")
            qm = mpool.tile([P, NT], F32, tag="qm")
            nc.sync.dma_start(qt[:], q_d[b].rearrange("(t p) d -> p t d", p=P))
            nc.sync.dma_start(ct[:], c_d[b].rearrange("(t p) d -> p t d", p=P))
            nc.sync.dma_start(kb[:], kb_d[b])
            nc.sync.dma_start(qm[:], qm_d[b])

            # ---- norms ----
            ssq = mpool.tile([P, 2 * NT], F32, tag="ssq")
            inv = mpool.tile([P, 2 * NT], F32, tag="inv")
            for t in range(NT):
                scr = spool.tile([P, D], F32, tag="scr")
                nc.vector.tensor_mul(scr[:], qt[:, t], qt[:, t])
                nc.vector.reduce_sum(ssq[:, t : t + 1], scr[:], axis=AX.X)
                scr2 = spool.tile([P, D], F32, tag="scr2")
                nc.scalar.activation(
                    scr2[:], ct[:, t], AF.Square,
                    accum_out=ssq[:, NT + t : NT + t + 1],
                )
            # inv = 1/sqrt(ssq)  (norms are >0 with randn inputs)
            nrm = mpool.tile([P, 2 * NT], F32, tag="nrm")
            nc.scalar.activation(nrm[:], ssq[:], AF.Sqrt)
            nc.vector.reciprocal(inv[:], nrm[:])

            # ---- qn in place, store first half of out ----
            for t in range(NT):
                nc.vector.tensor_scalar_mul(qt[:, t], qt[:, t], inv[:, t : t + 1])
            nc.sync.dma_start(
                out_d[b, :, 0:D].rearrange("(t p) d -> p t d", p=P), qt[:]
            )

            if PHASE < 2:
                continue
            # ---- transposes: qT[d, s] and cnT[d, s] ----
            qT = tpool.tile([P, ND, S], F32R, tag="qT")
            cT = tpool.tile([P, ND, S], F32R, tag="cT")
            for t in range(NT):
                pq = pst.tile([P, ND, P], F32, tag="pt")
                pc = pst.tile([P, ND, P], F32R, tag="pt")
                for dch in range(ND):
                    nc.tensor.transpose(
                        pq[:, dch], qt[:, t, dch * P : (dch + 1) * P], ident[:]
                    )
                    nc.tensor.transpose(
                        pc[:, dch], ct[:, t, dch * P : (dch + 1) * P], identr[:]
                    )
                nc.scalar.copy(qT[:, :, t * P : (t + 1) * P], pq[:])
                nc.vector.tensor_copy(cT[:, :, t * P : (t + 1) * P], pc[:])

            if PHASE < 3:
                continue
            # ---- mm1: sigT[k, q] = sigmoid(cnT.T @ qT + keybias) ----
            sg = sgpool.tile([P, NT, S], F32R, tag="sg")
            for kt in range(NT):
                for qc in range(2):
                    acc = ps1.tile([P, 512], F32, tag="acc")
                    for dch in range(ND):
                        nc.tensor.matmul(
                            acc[:],
                            cT[:, dch, kt * P : (kt + 1) * P],
                            qT[:, dch, qc * 512 : (qc + 1) * 512],
                            start=(dch == 0),
                            stop=(dch == ND - 1),
                        )
                    # context l2-normalization folds in as the per-k scale
                    nc.scalar.activation(
                        sg[:, kt, qc * 512 : (qc + 1) * 512], acc[:],
                        AF.Sigmoid, bias=kb[:, kt : kt + 1],
                        scale=inv[:, NT + kt : NT + kt + 1],
                    )

            if PHASE < 4:
                continue
            # ---- per q-block: denominator, attended, scores out ----
            for qb in range(NT):
                att = ps2.tile([P, 512], F32, tag="att")
                dn = psd.tile([P, 2], F32, tag="dn")
                for kt in range(NT):
                    sgblk = sg[:, kt, qb * P : (qb + 1) * P]
                    nc.tensor.matmul(
                        att[:], sgblk, ct[:, kt],
                        start=(kt == 0), stop=(kt == NT - 1),
                    )
                    nc.tensor.matmul(
                        dn[:], sgblk, ones[:],
                        start=(kt == 0), stop=(kt == NT - 1),
                    )
                # w = qmask / max(den, 1)
                w = mpool.tile([P, 1], F32, tag="w")
                nc.vector.tensor_scalar_max(w[:], dn[:, 0:1], 1.0)
                nc.vector.reciprocal(w[:], w[:])
                nc.vector.tensor_mul(w[:], w[:], qm[:, qb : qb + 1])

                ao = opool.tile([P, D], F32, tag="ao")
                nc.vector.tensor_scalar_mul(ao[:], att[:], w[:])
                nc.sync.dma_start(out_d[b, qb * P : (qb + 1) * P, D : 2 * D], ao[:])

                so = opool.tile([P, S], F32, tag="so")
                for kg in range(2):
                    pt = pst.tile([P, 4, P], F32R, tag="pt")
                    for j in range(4):
                        kt = kg * 4 + j
                        nc.tensor.transpose(
                            pt[:, j], sg[:, kt, qb * P : (qb + 1) * P], identr[:]
                        )
                    eng = nc.scalar if kg == 0 else nc.vector
                    if kg == 0:
                        nc.scalar.activation(
                            so[:, kg * 512 : (kg + 1) * 512], pt[:],
                            AF.Copy, scale=w[:],
                        )
                    else:
                        nc.vector.tensor_scalar_mul(
                            so[:, kg * 512 : (kg + 1) * 512], pt[:], w[:]
                        )
                nc.sync.dma_start(sc_d[b, qb * P : (qb + 1) * P, :], so[:])


_NC_CACHE = {}


def _get_nc():
    if "nc" not in _NC_CACHE:
        _NC_CACHE["nc"] = build_kernel()
    return _NC_CACHE["nc"]


def kernel(context, query, length):
    context = np.ascontiguousarray(np.asarray(context, dtype=np.float32))
    query = np.ascontiguousarray(np.asarray(query, dtype=np.float32))
    length = np.asarray(length).astype(np.int64)

    iot = np.arange(S)
    keymask = iot[None, :] < length[:, None]                      # [B, S]
    kbH = np.where(keymask, np.float32(0.0), NEG).astype(np.float32)
    kbH = np.ascontiguousarray(kbH.reshape(B, NT, P).transpose(0, 2, 1))
    qmH = keymask.astype(np.float32)
    qmH = np.ascontiguousarray(qmH.reshape(B, NT, P).transpose(0, 2, 1))
    ident = np.eye(P, dtype=np.float32)

    in_maps = []
    for c in range(NCORES):
        sl = slice(c * BPC, (c + 1) * BPC)
        in_maps.append(
            {
                "query": np.ascontiguousarray(query[sl]),
                "context": np.ascontiguousarray(context[sl]),
                "keybias": np.ascontiguousarray(kbH[sl]),
                "qmask": np.ascontiguousarray(qmH[sl]),
                "identity": ident,
                "identity_r": ident,
                "ones": np.ones((P, 2), dtype=np.float32),
            }
        )

    nc = _get_nc()
    res = run_bass_kernel_spmd(nc, in_maps, list(range(NCORES)))
    _NC_CACHE["last_result"] = res
    out = np.concatenate([res.results[c]["out"] for c in range(NCORES)], axis=0)
    scores = np.concatenate(
        [res.results[c]["scores"] for c in range(NCORES)], axis=0
    )
    return out, scores

